# revision 13
# baseline (speedup 1.0000x reference)
"""KNN graph kernel (DenseDilatedKnnGraph) for Trainium2, 8 NeuronCores.

Problem: x [2, 192, 8192, 1] fp32 -> edge_index [2, 2, 8192, 9] int32.
reference: L2-normalize x along C, pairwise sq-dists over N, top-9 (k=9,
dilation=1) nearest neighbors (indices), stacked with center indices.

Math: for normalized points, ranking by -dist == ranking by cosine
G = Xn^T Xn. Device computes, per query row, the comb-max vector
F2[p] = max_m G[q, p + 2048*m] (2048 combs of 4 columns) and ships F2
to the host. Host takes the top-24 combs per row (always contains every
comb holding a true top-9 column: a comb's max is >= the 9th value, and
device/host value skew is only the ~2e-4 fp16 rounding), rescores the
96 candidate columns exactly in fp64, and emits jax-top_k order.

Device schedule per 128-query row tile (~5.1-5.7 us per engine):
  - PE: Gram in 8 PSUM eighths [128, 1024] (bufs=4 -> depth-4
    pipeline). K=192 packed as a K=64 matmul pair row-tiled at array
    positions (0,0)/(64,0) running concurrently (weights+rhs duplicated
    across partition halves) plus a K=128 matmul, accumulating into the
    same PSUM banks. ~12.3k PE cycles/tile vs 16.4k for zero-padding.
  - ACT: evacuates eighths 0-3 and 7 to fp16 (5 ACTIVATEs).
  - DVE: eighths 4,5,6 fold directly from PSUM (tensor_max with one
    PSUM operand = fused evacuate+fold), 7 folds fp16x2, then two
    fp16 2x folds produce F2 [128, 2048].
  - DMA: F2 -> HBM (vout), 512 KB per tile.
"""

import numpy as np

B = 2
C = 192
N = 8192
NCORES = 8
RBLK = N // 4  # 2048 query rows per core
NT = RBLK // 128  # 16 row tiles per core
NV = 2048  # F2 width; comb(p) = {p + 2048*m : m = 0..3}
TCOMB = 24  # combs the host rescores per row

_cache = {}


def _build_nc():
    import concourse.bacc as bacc
    import concourse.mybir as mybir
    from concourse.bass import ts
    from concourse.tile import TileContext

    f32 = mybir.dt.float32
    f16 = mybir.dt.float16

    nc = bacc.Bacc("TRN2")

    # full (per-batch) normalized points + this core's query block
    xin = nc.dram_tensor("xin", [C, N], f16, kind="ExternalInput")
    wq = nc.dram_tensor("wq", [C, RBLK], f16, kind="ExternalInput")
    vout = nc.dram_tensor("vout", [RBLK, NV], f16, kind="ExternalOutput")

    DCH = 1024  # input DMA chunk

    with TileContext(nc) as tc:
        with (
            tc.tile_pool(name="xpool", bufs=1) as xpool,
            tc.tile_pool(name="gpool", bufs=3) as gpool,
            tc.tile_pool(name="fpool", bufs=3) as fpool,
            tc.tile_pool(name="vpool", bufs=3) as vpool,
            tc.tile_pool(name="gpsum", bufs=4, space="PSUM") as gpsum,
        ):
            # The 192 channels split into three 64-channel groups, each
            # duplicated across both SBUF partition halves so any matmul
            # can run on either PE row-group half.  All matmuls feeding
            # one PSUM eighth use the same half (eighth j -> half j%2):
            # same-bank matmuls serialize on the array half (immune to
            # scheduler reordering), opposite-half eighths overlap, and
            # every (half-array) LDWEIGHTS hides under the other half.
            hAq = xpool.tile([128, RBLK], f16)
            hCq = xpool.tile([128, RBLK], f16)
            hBq = xpool.tile([128, RBLK], f16)
            for pr in (slice(0, 64), slice(64, 128)):
                nc.sync.dma_start(hAq[pr, :], wq[0:64, :])
                nc.sync.dma_start(hCq[pr, :], wq[64:128, :])
                nc.sync.dma_start(hBq[pr, :], wq[128:192, :])

            hA = xpool.tile([128, N], f16)
            hC = xpool.tile([128, N], f16)
            hB = xpool.tile([128, N], f16)
            for pr in (slice(0, 64), slice(64, 128)):
                for sc in range(2):  # first chunk split for early start
                    ssl = ts(sc, 512)
                    nc.sync.dma_start(hA[pr, ssl], xin[0:64, ssl])
                    nc.sync.dma_start(hC[pr, ssl], xin[64:128, ssl])
                    nc.sync.dma_start(hB[pr, ssl], xin[128:192, ssl])
                for dc in range(1, N // DCH):
                    dsl = ts(dc, DCH)
                    nc.sync.dma_start(hA[pr, dsl], xin[0:64, dsl])
                    nc.sync.dma_start(hC[pr, dsl], xin[64:128, dsl])
                    nc.sync.dma_start(hB[pr, dsl], xin[128:192, dsl])

            for t in range(NT):
                tsl = ts(t, 128)
                a = {}
                for j in (0, 1, 2, 3, 7):
                    a[j] = gpool.tile(
                        [128, 1024], f16, tag=f"a{j}", name=f"a{j}"
                    )
                P = {}
                for j in range(4):
                    P[j] = fpool.tile(
                        [128, 1024], f16, tag=f"P{j}", name=f"P{j}"
                    )
                Fv = vpool.tile([128, NV], f16, tag="Fv")

                # All three K=64 matmuls of a chunk accumulate its bank
                # on the eighth's own array half.
                lo, hi = slice(0, 64), slice(64, 128)
                for pair in range(4):
                    psA = gpsum.tile([128, 1024], f32, tag="ps", name="psA")
                    psB = gpsum.tile([128, 1024], f32, tag="ps", name="psB")
                    for half, ps in ((0, psA), (1, psB)):
                        pr = lo if half == 0 else hi
                        for hh in range(2):
                            csl = ts(4 * pair + 2 * half + hh, 512)
                            osl = slice(512 * hh, 512 * hh + 512)
                            nc.tensor.matmul(
                                ps[:, osl], hAq[pr, tsl], hA[pr, csl],
                                start=True, stop=False,
                            )
                            nc.tensor.matmul(
                                ps[:, osl], hCq[pr, tsl], hC[pr, csl],
                                start=False, stop=False,
                            )
                            nc.tensor.matmul(
                                ps[:, osl], hBq[pr, tsl], hB[pr, csl],
                                start=False, stop=True,
                            )
                    # consumers, issued as soon as their eighth is done
                    if pair == 0:
                        nc.scalar.copy(a[0], psA)
                        nc.scalar.copy(a[1], psB)
                    elif pair == 1:
                        nc.scalar.copy(a[2], psA)
                        nc.scalar.copy(a[3], psB)
                    elif pair == 2:
                        nc.vector.tensor_max(P[0], psA, a[0])
                        nc.vector.tensor_max(P[1], psB, a[1])
                    else:
                        nc.vector.tensor_max(P[2], psA, a[2])
                        nc.scalar.copy(a[7], psB)
                        nc.vector.tensor_max(P[3], a[7], a[3])
                nc.vector.tensor_max(Fv[:, 0:1024], P[0], P[2])
                nc.vector.tensor_max(Fv[:, 1024:2048], P[1], P[3])
                nc.sync.dma_start(vout[tsl, :], Fv)

    nc.compile()
    return nc


def _get_nc():
    if "nc" not in _cache:
        _cache["nc"] = _build_nc()
    return _cache["nc"]


def shard_inputs(x):
    """x: [B, C, N, 1] -> 8 per-core inputs: normalized fp16 points
    (full batch) + the core's own 2048-column query block."""
    xs = np.ascontiguousarray(np.asarray(x, dtype=np.float32).reshape(B, C, N))
    rns = 1.0 / np.sqrt((xs * xs).sum(axis=1, keepdims=True))  # [B, 1, N]
    h16 = (xs * rns).astype(np.float16)
    in_maps = []
    for c in range(NCORES):
        b, r = divmod(c, 4)
        s = r * RBLK
        in_maps.append(
            {"xin": h16[b], "wq": np.ascontiguousarray(h16[b][:, s : s + RBLK])}
        )
    return in_maps


def assemble(results, x):
    """results: 8 dicts with 'vout' [RBLK, NV] f16 comb-max vectors.

    comb(p) = {p + 2048*m : m = 0..3}. Take top-TCOMB combs per row,
    rescore all TCOMB*4 candidate columns with exact fp64 dots of the
    normalized points, and take the true top-8 by (-value, index).
    """
    xs = np.asarray(x, dtype=np.float32).reshape(B, C, N)
    n64 = np.sqrt((xs.astype(np.float64) ** 2).sum(axis=1, keepdims=True))
    xn = np.ascontiguousarray((xs / n64).transpose(0, 2, 1))  # [B, N, C] f64

    nn = np.empty((B, N, 9), np.int32)
    m_off = (np.arange(4, dtype=np.int64) * NV)[None, None, :]
    for c in range(NCORES):
        b, r = divmod(c, 4)
        s = r * RBLK
        V = results[c]["vout"]  # [RBLK, NV] f16
        combs = np.argpartition(-V, TCOMB, axis=1)[:, :TCOMB].astype(np.int64)
        cand = (combs[:, :, None] + m_off).reshape(RBLK, TCOMB * 4)
        rows = np.arange(s, s + RBLK, dtype=np.int64)
        xnb = xn[b]
        top8 = np.empty((RBLK, 8), np.int64)
        CH = 512
        for r0 in range(0, RBLK, CH):
            cc = cand[r0 : r0 + CH]
            rr = rows[r0 : r0 + CH]
            vals = np.einsum("rkc,rc->rk", xnb[cc], xnb[rr], optimize=True)
            vals[cc == rr[:, None]] = -np.inf
            order = np.lexsort((cc, -vals), axis=-1)[:, :8]
            top8[r0 : r0 + CH] = np.take_along_axis(cc, order, axis=1)
        nn[b, s : s + RBLK, 1:9] = top8
        nn[b, s : s + RBLK, 0] = rows
    center = np.broadcast_to(np.arange(N, dtype=np.int32)[None, :, None], (B, N, 9))
    return np.ascontiguousarray(np.stack([nn, center], axis=0).astype(np.int32))


def kernel(x, _trace=False, **trace_kwargs):
    from concourse.bass_utils import run_bass_kernel_spmd

    nc = _get_nc()
    in_maps = shard_inputs(x)
    res = run_bass_kernel_spmd(
        nc, in_maps, core_ids=list(range(NCORES)), trace=_trace, **trace_kwargs
    )
    _cache["last_results"] = res
    return assemble(res.results, x)


# revision 15
# speedup vs baseline: 1.6836x; 1.6836x over previous
"""KNN graph kernel (DenseDilatedKnnGraph) for Trainium2, 8 NeuronCores.

Problem: x [2, 192, 8192, 1] fp32 -> edge_index [2, 2, 8192, 9] int32.
reference: L2-normalize x along C, pairwise sq-dists over N, top-9 (k=9,
dilation=1) nearest neighbors (indices), stacked with center indices.

Math: for normalized points, ranking by -dist == ranking by cosine
G = Xn^T Xn. Device computes, per query row, the comb-max vector
F2[p] = max_m G[q, p + 2048*m] (2048 combs of 4 columns) and ships F2
to the host. Host takes the top-32 combs per row (always contains every
comb holding a true top-9 column: a comb's max is >= the 9th value, and
the device/host value skew is the ~4e-3 fp8 input quantization),
rescores the 128 candidate columns exactly in fp64, and emits jax-top_k
order.

Device schedule per 128-query row tile:
  - PE: Gram in 8 PSUM eighths [128, 1024] (bufs=4 -> depth-4
    pipeline). fp8e4m3 DoubleRow packs K=192 (zero-padded to 256: two
    fp8 weights per PE cell) into ONE matmul per 512-column chunk --
    half the matmul passes and PSUM drain traffic of the fp16 version.
    Inputs are staged [128 partitions, 2, cols]: subtile 0 = channels
    0-127, subtile 1 = channels 128-191 (rows 64-127 zero).
  - ACT: evacuates eighths 0-3 and 7 to fp16 (5 ACTIVATEs, ~1.1us).
  - DVE: eighths 4,5,6 fold directly from PSUM (tensor_max with one
    PSUM operand = fused evacuate+fold), 7 folds fp16 2x, then two
    fp16 2x folds produce F2 [128, 2048].
  - DMA: F2 -> HBM (vout), 512 KB per tile.
"""

import numpy as np

B = 2
C = 192
N = 8192
NCORES = 8
RBLK = N // 4  # 2048 query rows per core
NT = RBLK // 128  # 16 row tiles per core
NV = 2048  # F2 width; comb(p) = {p + 2048*m : m = 0..3}
TCOMB = 32  # combs the host rescores per row

_cache = {}


def _build_nc():
    import concourse.bacc as bacc
    import concourse.mybir as mybir
    from concourse.bass import ts
    from concourse.tile import TileContext

    f32 = mybir.dt.float32
    f16 = mybir.dt.float16
    f8 = mybir.dt.float8e4

    nc = bacc.Bacc("TRN2")

    # fp8 points in DoubleRow layout [partition, k-subtile, col]:
    # [p, 0, n] = channel p, [p, 1, n] = channel 128+p (p<64, else 0).
    xin = nc.dram_tensor("xin", [128, 2, N], f8, kind="ExternalInput")
    wq = nc.dram_tensor("wq", [128, 2, RBLK], f8, kind="ExternalInput")
    vout = nc.dram_tensor("vout", [RBLK, NV], f16, kind="ExternalOutput")

    DCH = 2048  # input DMA chunk

    with TileContext(nc) as tc:
        with (
            tc.tile_pool(name="xpool", bufs=1) as xpool,
            tc.tile_pool(name="gpool", bufs=3) as gpool,
            tc.tile_pool(name="fpool", bufs=3) as fpool,
            tc.tile_pool(name="vpool", bufs=3) as vpool,
            tc.tile_pool(name="gpsum", bufs=4, space="PSUM") as gpsum,
        ):
            wqD = xpool.tile([128, 2, RBLK], f8)
            hD = xpool.tile([128, 2, N], f8)
            for s in range(2):
                nc.sync.dma_start(wqD[:, s, :], wq[:, s, :])
            for s in range(2):
                for sc in range(2):  # first chunk split for early start
                    ssl = ts(sc, DCH // 2)
                    nc.sync.dma_start(hD[:, s, ssl], xin[:, s, ssl])
                for dc in range(1, N // DCH):
                    dsl = ts(dc, DCH)
                    nc.sync.dma_start(hD[:, s, dsl], xin[:, s, dsl])

            for t in range(NT):
                tsl = ts(t, 128)
                a = {}
                for j in (0, 1, 2, 3, 7):
                    a[j] = gpool.tile(
                        [128, 1024], f16, tag=f"a{j}", name=f"a{j}"
                    )
                P = {}
                for j in range(4):
                    P[j] = fpool.tile(
                        [128, 1024], f16, tag=f"P{j}", name=f"P{j}"
                    )
                Fv = vpool.tile([128, NV], f16, tag="Fv")

                for pair in range(4):
                    psA = gpsum.tile([128, 1024], f32, tag="ps", name="psA")
                    psB = gpsum.tile([128, 1024], f32, tag="ps", name="psB")
                    for half, ps in ((0, psA), (1, psB)):
                        for hh in range(2):
                            csl = ts(4 * pair + 2 * half + hh, 512)
                            osl = slice(512 * hh, 512 * hh + 512)
                            nc.tensor.matmul(
                                ps[:, osl], wqD[:, :, tsl], hD[:, :, csl],
                                start=True, stop=True,
                                perf_mode=mybir.MatmulPerfMode.DoubleRow,
                            )
                    # consumers, issued as soon as their eighth is done
                    if pair == 0:
                        nc.scalar.copy(a[0], psA)
                        nc.scalar.copy(a[1], psB)
                    elif pair == 1:
                        nc.scalar.copy(a[2], psA)
                        nc.scalar.copy(a[3], psB)
                    elif pair == 2:
                        nc.vector.tensor_max(P[0], psA, a[0])
                        nc.vector.tensor_max(P[1], psB, a[1])
                    else:
                        nc.vector.tensor_max(P[2], psA, a[2])
                        nc.scalar.copy(a[7], psB)
                        nc.vector.tensor_max(P[3], a[7], a[3])
                nc.vector.tensor_max(Fv[:, 0:1024], P[0], P[2])
                nc.vector.tensor_max(Fv[:, 1024:2048], P[1], P[3])
                nc.sync.dma_start(vout[tsl, :], Fv)

    nc.compile()
    return nc


def _get_nc():
    if "nc" not in _cache:
        _cache["nc"] = _build_nc()
    return _cache["nc"]


def shard_inputs(x):
    """x: [B, C, N, 1] -> 8 per-core inputs: normalized fp8 points in
    DoubleRow layout (full batch) + the core's own query block."""
    import ml_dtypes

    f8 = ml_dtypes.float8_e4m3
    xs = np.ascontiguousarray(np.asarray(x, dtype=np.float32).reshape(B, C, N))
    rns = 1.0 / np.sqrt((xs * xs).sum(axis=1, keepdims=True))  # [B, 1, N]
    xn = xs * rns
    arr = np.zeros((B, 128, 2, N), dtype=f8)
    arr[:, :, 0, :] = xn[:, 0:128, :].astype(f8)
    arr[:, 0:64, 1, :] = xn[:, 128:192, :].astype(f8)
    in_maps = []
    for c in range(NCORES):
        b, r = divmod(c, 4)
        s = r * RBLK
        in_maps.append(
            {
                "xin": arr[b],
                "wq": np.ascontiguousarray(arr[b][:, :, s : s + RBLK]),
            }
        )
    return in_maps


def assemble(results, x):
    """results: 8 dicts with 'vout' [RBLK, NV] f16 comb-max vectors.

    comb(p) = {p + 2048*m : m = 0..3}. Take top-TCOMB combs per row,
    rescore all TCOMB*4 candidate columns with exact fp64 dots of the
    normalized points, and take the true top-8 by (-value, index).
    """
    xs = np.asarray(x, dtype=np.float32).reshape(B, C, N)
    n64 = np.sqrt((xs.astype(np.float64) ** 2).sum(axis=1, keepdims=True))
    xn = np.ascontiguousarray((xs / n64).transpose(0, 2, 1))  # [B, N, C] f64

    nn = np.empty((B, N, 9), np.int32)
    m_off = (np.arange(4, dtype=np.int64) * NV)[None, None, :]
    for c in range(NCORES):
        b, r = divmod(c, 4)
        s = r * RBLK
        V = results[c]["vout"]  # [RBLK, NV] f16
        combs = np.argpartition(-V, TCOMB, axis=1)[:, :TCOMB].astype(np.int64)
        cand = (combs[:, :, None] + m_off).reshape(RBLK, TCOMB * 4)
        rows = np.arange(s, s + RBLK, dtype=np.int64)
        xnb = xn[b]
        top8 = np.empty((RBLK, 8), np.int64)
        CH = 512
        for r0 in range(0, RBLK, CH):
            cc = cand[r0 : r0 + CH]
            rr = rows[r0 : r0 + CH]
            vals = np.einsum("rkc,rc->rk", xnb[cc], xnb[rr], optimize=True)
            vals[cc == rr[:, None]] = -np.inf
            order = np.lexsort((cc, -vals), axis=-1)[:, :8]
            top8[r0 : r0 + CH] = np.take_along_axis(cc, order, axis=1)
        nn[b, s : s + RBLK, 1:9] = top8
        nn[b, s : s + RBLK, 0] = rows
    center = np.broadcast_to(np.arange(N, dtype=np.int32)[None, :, None], (B, N, 9))
    return np.ascontiguousarray(np.stack([nn, center], axis=0).astype(np.int32))


def kernel(x, _trace=False, **trace_kwargs):
    from concourse.bass_utils import run_bass_kernel_spmd

    nc = _get_nc()
    in_maps = shard_inputs(x)
    res = run_bass_kernel_spmd(
        nc, in_maps, core_ids=list(range(NCORES)), trace=_trace, **trace_kwargs
    )
    _cache["last_results"] = res
    return assemble(res.results, x)


# revision 19
# speedup vs baseline: 1.6896x; 1.0036x over previous
"""KNN graph kernel (DenseDilatedKnnGraph) for Trainium2, 8 NeuronCores.

Problem: x [2, 192, 8192, 1] fp32 -> edge_index [2, 2, 8192, 9] int32.
reference: L2-normalize x along C, pairwise sq-dists over N, top-9 (k=9,
dilation=1) nearest neighbors (indices), stacked with center indices.

Math: for normalized points, ranking by -dist == ranking by cosine
G = Xn^T Xn. Device computes, per query row, the comb-max vector
F2[p] = max_m G[q, p + 2048*m] (2048 combs of 4 columns) and ships F2
to the host. Host takes the top-32 combs per row (always contains every
comb holding a true top-9 column: a comb's max is >= the 9th value, and
the device/host value skew is the ~4e-3 fp8 input quantization),
rescores the 128 candidate columns exactly in fp64, and emits jax-top_k
order.

Device schedule per 128-query row tile:
  - PE: Gram in 8 PSUM eighths [128, 1024] (bufs=4 -> depth-4
    pipeline). fp8e4m3 DoubleRow packs K=192 (zero-padded to 256: two
    fp8 weights per PE cell) into ONE matmul per 512-column chunk --
    half the matmul passes and PSUM drain traffic of the fp16 version.
    Inputs are staged [128 partitions, 2, cols]: subtile 0 = channels
    0-127, subtile 1 = channels 128-191 (rows 64-127 zero).
  - ACT: evacuates eighths 0-3 and 7 to fp16 (5 ACTIVATEs, ~1.1us).
  - DVE: eighths 4,5,6 fold directly from PSUM (tensor_max with one
    PSUM operand = fused evacuate+fold), 7 folds fp16 2x, then two
    fp16 2x folds produce F2 [128, 2048].
  - DMA: F2 -> HBM (vout), 512 KB per tile.
"""

import numpy as np

B = 2
C = 192
N = 8192
NCORES = 8
RBLK = N // 4  # 2048 query rows per core
NT = RBLK // 128  # 16 row tiles per core
NV = 4096  # F1 width; comb(p) = {p, p + 4096}
TCOMB = 64  # combs the host rescores per row

_cache = {}


def _build_nc():
    import concourse.bacc as bacc
    import concourse.mybir as mybir
    from concourse.bass import ts
    from concourse.tile import TileContext

    f32 = mybir.dt.float32
    f16 = mybir.dt.float16
    f8 = mybir.dt.float8e4

    nc = bacc.Bacc("TRN2")

    # fp8 points in DoubleRow layout [partition, k-subtile, col]:
    # [p, 0, n] = channel p, [p, 1, n] = channel 128+p (p<64, else 0).
    xin = nc.dram_tensor("xin", [128, 2, N], f8, kind="ExternalInput")
    wq = nc.dram_tensor("wq", [128, 2, RBLK], f8, kind="ExternalInput")
    vout = nc.dram_tensor("vout", [RBLK, NV], f16, kind="ExternalOutput")

    DCH = 2048  # input DMA chunk

    with TileContext(nc) as tc:
        with (
            tc.tile_pool(name="xpool", bufs=1) as xpool,
            tc.tile_pool(name="gpool", bufs=3) as gpool,
            tc.tile_pool(name="fpool", bufs=3) as fpool,
            tc.tile_pool(name="vpool", bufs=3) as vpool,
            tc.tile_pool(name="gpsum", bufs=4, space="PSUM") as gpsum,
        ):
            wqD = xpool.tile([128, 2, RBLK], f8)
            hD = xpool.tile([128, 2, N], f8)
            for s in range(2):
                nc.sync.dma_start(wqD[:, s, :], wq[:, s, :])
            for s in range(2):
                for sc in range(2):  # first chunk split for early start
                    ssl = ts(sc, DCH // 2)
                    nc.sync.dma_start(hD[:, s, ssl], xin[:, s, ssl])
                for dc in range(1, N // DCH):
                    dsl = ts(dc, DCH)
                    nc.sync.dma_start(hD[:, s, dsl], xin[:, s, dsl])

            for t in range(NT):
                tsl = ts(t, 128)
                a = {}
                for j in range(4):
                    a[j] = gpool.tile(
                        [128, 1024], f16, tag=f"a{j}", name=f"a{j}"
                    )
                F1 = fpool.tile([128, NV], f16, tag="F1")

                for pair in range(4):
                    psA = gpsum.tile([128, 1024], f32, tag="ps", name="psA")
                    psB = gpsum.tile([128, 1024], f32, tag="ps", name="psB")
                    for half, ps in ((0, psA), (1, psB)):
                        for hh in range(2):
                            csl = ts(4 * pair + 2 * half + hh, 512)
                            osl = slice(512 * hh, 512 * hh + 512)
                            nc.tensor.matmul(
                                ps[:, osl], wqD[:, :, tsl], hD[:, :, csl],
                                start=True, stop=True,
                                perf_mode=mybir.MatmulPerfMode.DoubleRow,
                            )
                    # eighths 0-3: ACT evacuates to fp16.  eighths 4-7:
                    # DVE folds straight from PSUM against the matching
                    # evacuated eighth (comb(d) = {d, d+4096}).
                    if pair == 0:
                        nc.scalar.copy(a[0], psA)
                        nc.scalar.copy(a[1], psB)
                    elif pair == 1:
                        nc.scalar.copy(a[2], psA)
                        nc.scalar.copy(a[3], psB)
                    elif pair == 2:
                        nc.vector.tensor_max(F1[:, 0:1024], psA, a[0])
                        nc.vector.tensor_max(F1[:, 1024:2048], psB, a[1])
                    else:
                        nc.vector.tensor_max(F1[:, 2048:3072], psA, a[2])
                        nc.vector.tensor_max(F1[:, 3072:4096], psB, a[3])
                nc.sync.dma_start(vout[tsl, :], F1)

    nc.compile()
    return nc


def _get_nc():
    if "nc" not in _cache:
        _cache["nc"] = _build_nc()
    return _cache["nc"]


def shard_inputs(x):
    """x: [B, C, N, 1] -> 8 per-core inputs: normalized fp8 points in
    DoubleRow layout (full batch) + the core's own query block."""
    import ml_dtypes

    f8 = ml_dtypes.float8_e4m3
    xs = np.ascontiguousarray(np.asarray(x, dtype=np.float32).reshape(B, C, N))
    rns = 1.0 / np.sqrt((xs * xs).sum(axis=1, keepdims=True))  # [B, 1, N]
    xn = xs * rns
    arr = np.zeros((B, 128, 2, N), dtype=f8)
    arr[:, :, 0, :] = xn[:, 0:128, :].astype(f8)
    arr[:, 0:64, 1, :] = xn[:, 128:192, :].astype(f8)
    in_maps = []
    for c in range(NCORES):
        b, r = divmod(c, 4)
        s = r * RBLK
        in_maps.append(
            {
                "xin": arr[b],
                "wq": np.ascontiguousarray(arr[b][:, :, s : s + RBLK]),
            }
        )
    return in_maps


def assemble(results, x):
    """results: 8 dicts with 'vout' [RBLK, NV] f16 comb-max vectors.

    comb(p) = {p + 2048*m : m = 0..3}. Take top-TCOMB combs per row,
    rescore all TCOMB*4 candidate columns with exact fp64 dots of the
    normalized points, and take the true top-8 by (-value, index).
    """
    xs = np.asarray(x, dtype=np.float32).reshape(B, C, N)
    n64 = np.sqrt((xs.astype(np.float64) ** 2).sum(axis=1, keepdims=True))
    xn = np.ascontiguousarray((xs / n64).transpose(0, 2, 1))  # [B, N, C] f64

    nn = np.empty((B, N, 9), np.int32)
    m_off = (np.arange(2, dtype=np.int64) * NV)[None, None, :]
    for c in range(NCORES):
        b, r = divmod(c, 4)
        s = r * RBLK
        V = results[c]["vout"]  # [RBLK, NV] f16
        combs = np.argpartition(-V, TCOMB, axis=1)[:, :TCOMB].astype(np.int64)
        cand = (combs[:, :, None] + m_off).reshape(RBLK, TCOMB * 2)
        rows = np.arange(s, s + RBLK, dtype=np.int64)
        xnb = xn[b]
        top8 = np.empty((RBLK, 8), np.int64)
        CH = 512
        for r0 in range(0, RBLK, CH):
            cc = cand[r0 : r0 + CH]
            rr = rows[r0 : r0 + CH]
            vals = np.einsum("rkc,rc->rk", xnb[cc], xnb[rr], optimize=True)
            vals[cc == rr[:, None]] = -np.inf
            order = np.lexsort((cc, -vals), axis=-1)[:, :8]
            top8[r0 : r0 + CH] = np.take_along_axis(cc, order, axis=1)
        nn[b, s : s + RBLK, 1:9] = top8
        nn[b, s : s + RBLK, 0] = rows
    center = np.broadcast_to(np.arange(N, dtype=np.int32)[None, :, None], (B, N, 9))
    return np.ascontiguousarray(np.stack([nn, center], axis=0).astype(np.int32))


def kernel(x, _trace=False, **trace_kwargs):
    from concourse.bass_utils import run_bass_kernel_spmd

    nc = _get_nc()
    in_maps = shard_inputs(x)
    res = run_bass_kernel_spmd(
        nc, in_maps, core_ids=list(range(NCORES)), trace=_trace, **trace_kwargs
    )
    _cache["last_results"] = res
    return assemble(res.results, x)


# revision 20
# speedup vs baseline: 1.8896x; 1.1183x over previous
"""KNN graph kernel (DenseDilatedKnnGraph) for Trainium2, 8 NeuronCores.

Problem: x [2, 192, 8192, 1] fp32 -> edge_index [2, 2, 8192, 9] int32.
reference: L2-normalize x along C, pairwise sq-dists over N, top-9 (k=9,
dilation=1) nearest neighbors (indices), stacked with center indices.

Math: for normalized points, ranking by -dist == ranking by cosine
G = Xn^T Xn. Device computes, per query row, the comb-max vector
F2[p] = max_m G[q, p + 2048*m] (2048 combs of 4 columns) and ships F2
to the host. Host takes the top-32 combs per row (always contains every
comb holding a true top-9 column: a comb's max is >= the 9th value, and
the device/host value skew is the ~4e-3 fp8 input quantization),
rescores the 128 candidate columns exactly in fp64, and emits jax-top_k
order.

Device schedule per 128-query row tile:
  - PE: Gram in 8 PSUM eighths [128, 1024] (bufs=4 -> depth-4
    pipeline). fp8e4m3 DoubleRow packs K=192 (zero-padded to 256: two
    fp8 weights per PE cell) into ONE matmul per 512-column chunk --
    half the matmul passes and PSUM drain traffic of the fp16 version.
    Inputs are staged [128 partitions, 2, cols]: subtile 0 = channels
    0-127, subtile 1 = channels 128-191 (rows 64-127 zero).
  - ACT: evacuates eighths 0-3 and 7 to fp16 (5 ACTIVATEs, ~1.1us).
  - DVE: eighths 4,5,6 fold directly from PSUM (tensor_max with one
    PSUM operand = fused evacuate+fold), 7 folds fp16 2x, then two
    fp16 2x folds produce F2 [128, 2048].
  - DMA: F2 -> HBM (vout), 512 KB per tile.
"""

import numpy as np

B = 2
C = 192
N = 8192
NCORES = 8
RBLK = N // 4  # 2048 query rows per core
NT = RBLK // 128  # 16 row tiles per core
NV = 4096  # F1 width; comb(p) = {p, p + 4096}
TCOMB = 64  # combs the host rescores per row

_cache = {}


def _build_nc():
    import concourse.bacc as bacc
    import concourse.mybir as mybir
    from concourse.bass import ts
    from concourse.tile import TileContext

    f32 = mybir.dt.float32
    f16 = mybir.dt.float16
    f8 = mybir.dt.float8e4

    nc = bacc.Bacc("TRN2")

    # fp8 points in DoubleRow layout [partition, k-subtile, col]:
    # [p, 0, n] = channel p, [p, 1, n] = channel 128+p (p<64, else 0).
    xin = nc.dram_tensor("xin", [128, 2, N], f8, kind="ExternalInput")
    wq = nc.dram_tensor("wq", [128, 2, RBLK], f8, kind="ExternalInput")
    vout = nc.dram_tensor("vout", [RBLK, NV], f16, kind="ExternalOutput")

    DCH = 2048  # input DMA chunk

    with TileContext(nc) as tc:
        with (
            tc.tile_pool(name="xpool", bufs=1) as xpool,
            tc.tile_pool(name="gpool", bufs=3) as gpool,
            tc.tile_pool(name="fpool", bufs=3) as fpool,
            tc.tile_pool(name="vpool", bufs=3) as vpool,
            tc.tile_pool(name="gpsum", bufs=4, space="PSUM") as gpsum,
        ):
            wqD = xpool.tile([128, 2, RBLK], f8)
            hD = xpool.tile([128, 2, N], f8)
            for s in range(2):
                nc.sync.dma_start(wqD[:, s, :], wq[:, s, :])
            for s in range(2):
                for sc in range(2):  # first chunk split for early start
                    ssl = ts(sc, DCH // 2)
                    nc.sync.dma_start(hD[:, s, ssl], xin[:, s, ssl])
                for dc in range(1, N // DCH):
                    dsl = ts(dc, DCH)
                    nc.sync.dma_start(hD[:, s, dsl], xin[:, s, dsl])

            for t in range(NT):
                tsl = ts(t, 128)
                a = {}
                for j in range(4):
                    a[j] = gpool.tile(
                        [128, 1024], f16, tag=f"a{j}", name=f"a{j}"
                    )
                F1 = fpool.tile([128, NV], f16, tag="F1")

                # Eighths j and j+4 are computed together: ACT
                # evacuates eighth j to fp16, DVE folds eighth j+4
                # straight from PSUM against it (comb(d) = {d, d+4096}).
                # Alternating the two consumers per pair keeps ACT and
                # DVE streaming concurrently instead of phase-locking
                # on the 4-deep PSUM pool.
                for pair in range(4):
                    psA = gpsum.tile([128, 1024], f32, tag="ps", name="psA")
                    psB = gpsum.tile([128, 1024], f32, tag="ps", name="psB")
                    for ps, j in ((psA, pair), (psB, pair + 4)):
                        for hh in range(2):
                            csl = ts(2 * j + hh, 512)
                            osl = slice(512 * hh, 512 * hh + 512)
                            nc.tensor.matmul(
                                ps[:, osl], wqD[:, :, tsl], hD[:, :, csl],
                                start=True, stop=True,
                                perf_mode=mybir.MatmulPerfMode.DoubleRow,
                            )
                    nc.scalar.copy(a[pair], psA)
                    nc.vector.tensor_max(
                        F1[:, 1024 * pair : 1024 * (pair + 1)], psB, a[pair]
                    )
                nc.sync.dma_start(vout[tsl, :], F1)

    nc.compile()
    return nc


def _get_nc():
    if "nc" not in _cache:
        _cache["nc"] = _build_nc()
    return _cache["nc"]


def shard_inputs(x):
    """x: [B, C, N, 1] -> 8 per-core inputs: normalized fp8 points in
    DoubleRow layout (full batch) + the core's own query block."""
    import ml_dtypes

    f8 = ml_dtypes.float8_e4m3
    xs = np.ascontiguousarray(np.asarray(x, dtype=np.float32).reshape(B, C, N))
    rns = 1.0 / np.sqrt((xs * xs).sum(axis=1, keepdims=True))  # [B, 1, N]
    xn = xs * rns
    arr = np.zeros((B, 128, 2, N), dtype=f8)
    arr[:, :, 0, :] = xn[:, 0:128, :].astype(f8)
    arr[:, 0:64, 1, :] = xn[:, 128:192, :].astype(f8)
    in_maps = []
    for c in range(NCORES):
        b, r = divmod(c, 4)
        s = r * RBLK
        in_maps.append(
            {
                "xin": arr[b],
                "wq": np.ascontiguousarray(arr[b][:, :, s : s + RBLK]),
            }
        )
    return in_maps


def assemble(results, x):
    """results: 8 dicts with 'vout' [RBLK, NV] f16 comb-max vectors.

    comb(p) = {p + 2048*m : m = 0..3}. Take top-TCOMB combs per row,
    rescore all TCOMB*4 candidate columns with exact fp64 dots of the
    normalized points, and take the true top-8 by (-value, index).
    """
    xs = np.asarray(x, dtype=np.float32).reshape(B, C, N)
    n64 = np.sqrt((xs.astype(np.float64) ** 2).sum(axis=1, keepdims=True))
    xn = np.ascontiguousarray((xs / n64).transpose(0, 2, 1))  # [B, N, C] f64

    nn = np.empty((B, N, 9), np.int32)
    m_off = (np.arange(2, dtype=np.int64) * NV)[None, None, :]
    for c in range(NCORES):
        b, r = divmod(c, 4)
        s = r * RBLK
        V = results[c]["vout"]  # [RBLK, NV] f16
        combs = np.argpartition(-V, TCOMB, axis=1)[:, :TCOMB].astype(np.int64)
        cand = (combs[:, :, None] + m_off).reshape(RBLK, TCOMB * 2)
        rows = np.arange(s, s + RBLK, dtype=np.int64)
        xnb = xn[b]
        top8 = np.empty((RBLK, 8), np.int64)
        CH = 512
        for r0 in range(0, RBLK, CH):
            cc = cand[r0 : r0 + CH]
            rr = rows[r0 : r0 + CH]
            vals = np.einsum("rkc,rc->rk", xnb[cc], xnb[rr], optimize=True)
            vals[cc == rr[:, None]] = -np.inf
            order = np.lexsort((cc, -vals), axis=-1)[:, :8]
            top8[r0 : r0 + CH] = np.take_along_axis(cc, order, axis=1)
        nn[b, s : s + RBLK, 1:9] = top8
        nn[b, s : s + RBLK, 0] = rows
    center = np.broadcast_to(np.arange(N, dtype=np.int32)[None, :, None], (B, N, 9))
    return np.ascontiguousarray(np.stack([nn, center], axis=0).astype(np.int32))


def kernel(x, _trace=False, **trace_kwargs):
    from concourse.bass_utils import run_bass_kernel_spmd

    nc = _get_nc()
    in_maps = shard_inputs(x)
    res = run_bass_kernel_spmd(
        nc, in_maps, core_ids=list(range(NCORES)), trace=_trace, **trace_kwargs
    )
    _cache["last_results"] = res
    return assemble(res.results, x)
